# revision 26
# baseline (speedup 1.0000x reference)
"""Self-contained Trainium2 Bass kernel: 16-head attention with RoPE (B=2, S=2048, D=2048).

Sharding: 8 cores = 2 (batch) x 4 (head groups of 4 heads / 512 cols).
No collectives: the output projection is row-split (each core multiplies its
own 4 heads' attention output X_g by Wo rows for those columns) and emits a
full-width PARTIAL output [D, S] bf16; the host sums the 4 partials per batch.

The kernel is a software pipeline over 4 q-chunks of 512. Emission interleaves
the previous chunk's output-projection groups into the current chunk's
projection phase and attention rounds so the PE never starves while the
exp/rowsum chain (ACT/DVE) runs.

Dataflow is fully "transposed" so no on-chip transposes are needed:
  hiddenT [d, s] (host-pretransposed, bf16), streamed per chunk
  QT/KT   [dh, s] per head  (projection emits head-dim-major directly)
  S^T     [k, q] scores, two heads packed in one [128, 2, 512] PSUM tile
  causal mask on diagonal tiles applied INSIDE the score accumulation group
          via an extra matmul: ss += triT.T @ I  (no vector op on that path)
  P^T     [k, q] = exp(S^T)            (one ACT exp per head-pair per k-tile)
  colsums accumulated on DVE (bf16 2x), partition-reduced by a ones-matmul
  O^T     [dh, q] = V^T @ P^T          (lhsT = V natural [s, dh])
  partial out^T [oc, q] = Wo[:, own].T @ O^T  (accumulated over own heads)
RoPE de-interleave is folded into a host-side row permutation of Wq/Wk;
RoPE itself is 3 bf16 2x-mode DVE ops using [cos;cos] / [-sin;sin] tables.
1/sqrt(DH) is folded into the Q rope tables. bq/bk applied via ACT bias
(per-partition); bv/bo folded into the output on the host.
"""

import math
from contextlib import ExitStack

import numpy as np
import ml_dtypes

B, S, D, H, DH = 2, 2048, 2048, 16, 128
NCORES = 8
GPC = 4            # cores per head-group dimension
HPC = H // GPC     # heads per core (4)
CW = HPC * DH      # 512 columns per core
NEG = -1e9
BF = ml_dtypes.bfloat16
QCH = 512          # q-chunk (moving free dim)
NQC = S // QCH     # 4
NDT = D // 128     # 16 d-tiles
NST = S // 128     # 16 s-tiles

_built = {}


def _build():
    import concourse.bass as bass
    import concourse.tile as tile
    from concourse import bacc, mybir

    f32, bf16 = mybir.dt.float32, mybir.dt.bfloat16
    EXP = mybir.ActivationFunctionType.Exp
    IDN = mybir.ActivationFunctionType.Identity
    CPY = mybir.ActivationFunctionType.Copy

    nc = bacc.Bacc("TRN2", target_bir_lowering=False, debug=False,
                   num_devices=NCORES)

    hT_d = nc.dram_tensor("hiddenT", [D, S], bf16, kind="ExternalInput")
    wq_d = nc.dram_tensor("wqT", [D, CW], bf16, kind="ExternalInput")
    wk_d = nc.dram_tensor("wkT", [D, CW], bf16, kind="ExternalInput")
    wv_d = nc.dram_tensor("wvT", [D, CW], bf16, kind="ExternalInput")
    wo_d = nc.dram_tensor("wopT", [CW, D], bf16, kind="ExternalInput")
    cq_d = nc.dram_tensor("cqs", [128, S], bf16, kind="ExternalInput")
    sq_d = nc.dram_tensor("sqs", [128, S], bf16, kind="ExternalInput")
    ck_d = nc.dram_tensor("cks", [128, S], bf16, kind="ExternalInput")
    sk_d = nc.dram_tensor("sks", [128, S], bf16, kind="ExternalInput")
    bq_d = nc.dram_tensor("bqp", [128, HPC], f32, kind="ExternalInput")
    bk_d = nc.dram_tensor("bkp", [128, HPC], f32, kind="ExternalInput")
    dm_d = nc.dram_tensor("dmaskT", [128, 128], bf16, kind="ExternalInput")
    id_d = nc.dram_tensor("ident", [128, 128], bf16, kind="ExternalInput")
    out_d = nc.dram_tensor("pout", [D, S], bf16, kind="ExternalOutput")

    with tile.TileContext(nc) as tc, ExitStack() as ctx:
        wp = ctx.enter_context(tc.tile_pool(name="wp", bufs=3 * NDT))
        wop = ctx.enter_context(tc.tile_pool(name="wop", bufs=HPC))
        hp = ctx.enter_context(tc.tile_pool(name="hp", bufs=2 * NDT))
        cst = ctx.enter_context(tc.tile_pool(name="cst", bufs=1))
        qkp = ctx.enter_context(tc.tile_pool(name="qkp", bufs=2 * HPC + 1))
        vp = ctx.enter_context(tc.tile_pool(name="vp", bufs=NST))
        rp = ctx.enter_context(tc.tile_pool(name="rp", bufs=4))
        ptp = ctx.enter_context(tc.tile_pool(name="ptp", bufs=8))
        rcp = ctx.enter_context(tc.tile_pool(name="rcp", bufs=2))
        otp = ctx.enter_context(tc.tile_pool(name="otp", bufs=10))
        ofp = ctx.enter_context(tc.tile_pool(name="ofp", bufs=4))
        ps_mm = ctx.enter_context(tc.tile_pool(name="ps_mm", bufs=2, space="PSUM"))
        ps_ss = ctx.enter_context(tc.tile_pool(name="ps_ss", bufs=2, space="PSUM"))
        ps_pv = ctx.enter_context(tc.tile_pool(name="ps_pv", bufs=2, space="PSUM"))

        # All bulk input loads go on the SP HWDGE queue: spreading them onto
        # the ACT queue delays the latency-critical projection drain copies
        # behind ~0.6us-per-trigger dispatch on the Scalar sequencer.
        def ldma(dst, src):
            nc.sync.dma_start(dst, src)

        # ---- first-needed data first: Wv + hT(chunk 0) interleaved ----
        wv_sb, wq_sb, wk_sb, wo_sb = [], [], [], []
        hTc0 = []
        # Startup dispatch is ~0.6us/trigger serial per queue, so split the
        # two initial streams across BOTH HWDGE queues (ACT is idle until the
        # first projection drain at ~20us).
        for dt in range(NDT):
            w = wp.tile([128, CW], bf16, tag="w", name=f"wv{dt}")
            eng = nc.sync if dt % 2 == 0 else nc.scalar
            eng.dma_start(w[:], wv_d[dt * 128:(dt + 1) * 128, :])
            wv_sb.append(w)
            t = hp.tile([128, QCH], bf16, tag="hT", name=f"hT0_{dt}")
            eng2 = nc.scalar if dt % 2 == 0 else nc.sync
            eng2.dma_start(t[:], hT_d[dt * 128:(dt + 1) * 128, 0:QCH])
            hTc0.append(t)
        # ---- constants / Q then K weights ----
        cq_sb = cst.tile([128, S], bf16, tag="cq", name="cq_sb")
        sq_sb = cst.tile([128, S], bf16, tag="sq", name="sq_sb")
        ck_sb = cst.tile([128, S], bf16, tag="ck", name="ck_sb")
        sk_sb = cst.tile([128, S], bf16, tag="sk", name="sk_sb")
        bq_sb = cst.tile([128, HPC], f32, tag="bq", name="bq_sb")
        bk_sb = cst.tile([128, HPC], f32, tag="bk", name="bk_sb")
        for dt in range(NDT):
            w = wp.tile([128, CW], bf16, tag="w", name=f"wq{dt}")
            ldma(w[:], wq_d[dt * 128:(dt + 1) * 128, :])
            wq_sb.append(w)
        ldma(cq_sb[:], cq_d[:])
        ldma(sq_sb[:], sq_d[:])
        ldma(bq_sb[:], bq_d[:])
        for dt in range(NDT):
            w = wp.tile([128, CW], bf16, tag="w", name=f"wk{dt}")
            ldma(w[:], wk_d[dt * 128:(dt + 1) * 128, :])
            wk_sb.append(w)
        ldma(ck_sb[:], ck_d[:])
        ldma(sk_sb[:], sk_d[:])
        ldma(bk_sb[:], bk_d[:])
        triT_sb = cst.tile([128, 128], bf16, tag="triT", name="triT_sb")
        ldma(triT_sb[:], dm_d[:])
        id_sb = cst.tile([128, 128], bf16, tag="ident", name="id_sb")
        ldma(id_sb[:], id_d[:])
        ones_sb = cst.tile([128, 128], bf16, tag="ones", name="ones_sb")
        nc.vector.memset(ones_sb[:], 1.0)
        # hT(1) after the chunk-0 weights: needed from ~50us so proj(1) can
        # fill attention(0) gaps, but must not delay wq/wk
        hTc1 = []
        for dt in range(NDT):
            t = hp.tile([128, QCH], bf16, tag="hT", name=f"hT1_{dt}")
            ldma(t[:], hT_d[dt * 128:(dt + 1) * 128, QCH:2 * QCH])
            hTc1.append(t)
        # Wo streams in behind everything else (needed first at outproj(0))
        for h in range(HPC):
            t = wop.tile([128, D], bf16, tag="wo", name=f"wo{h}")
            ldma(t[:], wo_d[h * 128:(h + 1) * 128, :])
            wo_sb.append(t)

        # persistent KT (written chunk by chunk; all history needed) and V;
        # QT is per-chunk only
        ktr = [qkp.tile([128, S], bf16, tag="ktr", name=f"ktr{m}", bufs=HPC)
               for m in range(HPC)]
        v_sb = [None] * NST
        hTcs = {0: hTc0, 1: hTc1}
        drain_flip = [0]

        def drain(dst, src):
            # alternate PSUM->SBUF drains between DVE and ACT so neither
            # engine serializes the psum slot recycling
            drain_flip[0] ^= 1
            if drain_flip[0]:
                nc.vector.tensor_copy(dst, src)
            else:
                nc.scalar.activation(dst, src, CPY)

        def rope_head(w_sb, b_sb, c_sb, s_sb, dst, dsl, hTc, c, m, prefix):
            """Project head m of chunk c and write RoPE'd rows to dst[:, dsl]."""
            csl = slice(c * QCH, (c + 1) * QCH)
            ps = ps_mm.tile([128, QCH], f32, tag="mm", name=f"{prefix}ps{m}_{c}")
            for dt in range(NDT):
                nc.tensor.matmul(ps[:], w_sb[dt][:, m * 128:(m + 1) * 128],
                                 hTc[dt][:],
                                 start=(dt == 0), stop=(dt == NDT - 1))
            raw = rp.tile([128, QCH], bf16, tag="raw", name=f"{prefix}rw{m}_{c}")
            # alternate the raw copy between ACT and DVE (both apply the
            # per-partition bias) so neither sequencer saturates
            drain_flip[0] ^= 1
            if drain_flip[0]:
                nc.scalar.activation(raw[:], ps[:], IDN, bias=b_sb[:, m:m + 1])
            else:
                nc.vector.tensor_scalar_add(raw[:], ps[:], b_sb[:, m:m + 1])
            t1 = rp.tile([128, QCH], bf16, tag="t1", name=f"{prefix}t1{m}_{c}")
            # half-swap on the ACT HWDGE queue: tiny latency-critical copies
            # must not sit behind bulk weight/hT transfers on the SP queue
            nc.scalar.dma_start(t1[0:64, :], raw[64:128, :])
            nc.scalar.dma_start(t1[64:128, :], raw[0:64, :])
            # dst = raw*[cos;cos] + swap(raw)*[-sin;sin]; the sin-multiply
            # runs on the otherwise-idle GPSIMD to decongest DVE
            nc.vector.tensor_mul(dst[:, dsl], raw[:], c_sb[:, csl])
            nc.gpsimd.tensor_mul(t1[:], t1[:], s_sb[:, csl])
            nc.vector.tensor_add(dst[:, dsl], dst[:, dsl], t1[:])

        def proj_tasks(c):
            """12 emission tasks: V s-tiles, Q heads (+rope), K heads (+rope)."""
            hTc = hTcs[c]
            tasks = []

            def v_task(sti):
                def go():
                    st = 4 * c + sti
                    ps = ps_mm.tile([128, CW], f32, tag="mm", name=f"psv{st}")
                    for dt in range(NDT):
                        nc.tensor.matmul(ps[:],
                                         hTc[dt][:, sti * 128:(sti + 1) * 128],
                                         wv_sb[dt][:],
                                         start=(dt == 0), stop=(dt == NDT - 1))
                    vt = vp.tile([128, CW], bf16, tag="v", name=f"v{st}")
                    drain(vt[:], ps[:])
                    v_sb[st] = vt
                return go

            qtrc = [qkp.tile([128, QCH], bf16, tag="qtc", name=f"qtc{c}_{m}",
                             bufs=HPC + 4) for m in range(HPC)]

            def q_task(m):
                return lambda: rope_head(wq_sb, bq_sb, cq_sb, sq_sb, qtrc[m],
                                         slice(0, QCH), hTc, c, m, "q")

            def k_task(m):
                return lambda: rope_head(wk_sb, bk_sb, ck_sb, sk_sb, ktr[m],
                                         slice(c * QCH, (c + 1) * QCH),
                                         hTc, c, m, "k")

            # pre: everything attention pass A (heads 0,1) needs.
            # mid: heads 2,3's projections — they become pass A's fillers so
            # their rope chains overlap pass A instead of gating it.
            for sti in range(4):
                tasks.append(v_task(sti))
            tasks.append(q_task(0))
            tasks.append(q_task(1))
            tasks.append(k_task(0))
            tasks.append(k_task(1))
            mid = [q_task(2), q_task(3), k_task(2), k_task(3)]
            return qtrc, tasks, mid

        def attention_pass(c, qtrc, pair, fillers):
            """Heads (2*pair, 2*pair+1) of chunk c; returns their ot tiles.
            Pops one filler emission task per k-tile round (if any left)."""
            nk = 4 * c + 4
            heads = (2 * pair, 2 * pair + 1)
            pv = {}
            for h in heads:
                pv[h] = ps_pv.tile([128, QCH], f32, tag="pv",
                                   name=f"pv{c}_{h}")
            sacc = ptp.tile([128, 2, QCH], bf16, tag="sacc",
                            name=f"sacc{c}_{pair}", bufs=3)
            prev_pt, prev_ki = None, None
            for ki in range(nk):
                p = ki - 4 * c
                c0 = max(0, 128 * p)
                ss = ps_ss.tile([128, 2, QCH], f32, tag="ss",
                                name=f"ss{c}_{pair}_{ki}")
                for j, h in enumerate(heads):
                    nc.tensor.matmul(ss[:, j, c0:],
                                     ktr[h][:, ki * 128:(ki + 1) * 128],
                                     qtrc[h][:, c0:],
                                     start=True, stop=(p < 0))
                    if p >= 0:
                        # causal mask inside the accumulation group:
                        # ss_band += triT.T @ I  (keeps ACT chain PE-only)
                        nc.tensor.matmul(ss[:, j, c0:c0 + 128], triT_sb[:],
                                         id_sb[:], start=False, stop=True)
                pt = ptp.tile([128, 2, QCH], bf16, tag="pt",
                              name=f"pt{c}_{pair}_{ki}", bufs=3)
                if c0 > 0:
                    nc.gpsimd.memset(pt[:, :, 0:c0], 0.0)
                nc.scalar.activation(pt[:, :, c0:], ss[:, :, c0:], EXP)
                if ki == 0:
                    nc.vector.tensor_copy(sacc[:], pt[:])
                else:
                    nc.vector.tensor_add(sacc[:, :, c0:], sacc[:, :, c0:],
                                         pt[:, :, c0:])
                # pv for the previous k-tile (one behind, so PE never waits
                # on the exp chain)
                if prev_pt is not None:
                    pc0 = max(0, 128 * (prev_ki - 4 * c))
                    for j, h in enumerate(heads):
                        nc.tensor.matmul(pv[h][:, pc0:],
                                         v_sb[prev_ki][:, h * 128:(h + 1) * 128],
                                         prev_pt[:, j, pc0:],
                                         start=(prev_ki == 0), stop=False)
                prev_pt, prev_ki = pt, ki
                if fillers:
                    fillers.pop(0)()
            fc0 = max(0, 128 * (prev_ki - 4 * c)) if prev_ki != 0 else 0
            for j, h in enumerate(heads):
                nc.tensor.matmul(pv[h][:, fc0:],
                                 v_sb[prev_ki][:, h * 128:(h + 1) * 128],
                                 prev_pt[:, j, fc0:],
                                 start=(prev_ki == 0), stop=True)
            # partition-reduce+broadcast the colsums (two 512-wide matmuls)
            sm = ps_ss.tile([128, 2, QCH], f32, tag="ss", name=f"sm{c}_{pair}")
            for j in range(2):
                nc.tensor.matmul(sm[:, j, :], ones_sb[:], sacc[:, j, :],
                                 start=True, stop=True)
            recb = rcp.tile([128, 2, QCH], f32, tag="recb",
                            name=f"recb{c}_{pair}")
            nc.vector.reciprocal_approx_fast(out=recb[:], in_=sm[:])
            ots = []
            for j, h in enumerate(heads):
                ot = otp.tile([128, QCH], bf16, tag="ot", name=f"ot{c}_{h}")
                nc.vector.tensor_mul(ot[:], pv[h][:], recb[:, j, :])
                ots.append(ot)
            return ots

        def outproj_tasks(c, ots, tail=False):
            """16 emission tasks, one [128,512] output tile each."""
            csl = slice(c * QCH, (c + 1) * QCH)

            def task(t):
                def go():
                    pool = ps_ss if (tail and t % 2) else ps_mm
                    po = pool.tile([128, QCH], f32,
                                   tag="ss" if (tail and t % 2) else "mm",
                                   name=f"po{c}_{t}")
                    for h in range(HPC):
                        nc.tensor.matmul(po[:],
                                         wo_sb[h][:, t * 128:(t + 1) * 128],
                                         ots[h][:],
                                         start=(h == 0), stop=(h == HPC - 1))
                    of = ofp.tile([128, QCH], bf16, tag="of", name=f"of{c}_{t}")
                    drain(of[:], po[:])
                    nc.sync.dma_start(out_d[t * 128:(t + 1) * 128, csl], of[:])
                return go
            return [task(t) for t in range(NDT)]

        # ---- main pipeline ----
        po_backlog = []   # outproj tasks of the previous chunk
        for c in range(NQC):
            if 1 <= c and c + 1 < NQC:
                nsl = slice((c + 1) * QCH, (c + 2) * QCH)
                nxt = []
                for dt in range(NDT):
                    t = hp.tile([128, QCH], bf16, tag="hT",
                                name=f"hT{c + 1}_{dt}")
                    ldma(t[:], hT_d[dt * 128:(dt + 1) * 128, nsl])
                    nxt.append(t)
                hTcs[c + 1] = nxt
            qtrc, ptasks, mid = proj_tasks(c)
            # interleave proj(c) groups with half of outproj(c-1) groups
            first_po = po_backlog[:8]
            rest_po = po_backlog[8:]
            k = 0
            for i, t in enumerate(ptasks):
                t()
                if k < len(first_po):
                    first_po[k]()
                    k += 1
            for t in first_po[k:]:
                t()
            # heads 2,3's projections + remaining outproj(c-1) groups fill
            # the attention rounds (pass A always has >= 4 rounds, so `mid`
            # is fully emitted before pass B needs those heads); hold a few
            # outproj groups back so pass B doesn't run dry
            fill_a = mid + rest_po[:4]
            fill_b = rest_po[4:]
            ots = attention_pass(c, qtrc, 0, fill_a)
            fill_b = fill_a + fill_b
            ots = ots + attention_pass(c, qtrc, 1, fill_b)
            for t in fill_b:
                t()
            po_backlog = outproj_tasks(c, ots, tail=(c == NQC - 1))
        for t in po_backlog:
            t()

    nc.compile()
    return nc


def _get_built():
    if "k" not in _built:
        _built["k"] = _build()
    return _built["k"]


def _prep_inputs(inputs):
    hs = np.asarray(inputs["hidden_states"], np.float32)
    fc = np.asarray(inputs["freqs_cis"], np.float32)
    Wq = np.asarray(inputs["Wq"], np.float32)
    Wk = np.asarray(inputs["Wk"], np.float32)
    Wv = np.asarray(inputs["Wv"], np.float32)
    Wo = np.asarray(inputs["Wo"], np.float32)
    bq = np.asarray(inputs["bq"], np.float32)
    bk = np.asarray(inputs["bk"], np.float32)

    # de-interleave permutation per 128-row head block: [0,2,..,126, 1,3,..,127]
    perm1 = np.concatenate([np.arange(0, DH, 2), np.arange(1, DH, 2)])
    permC = (np.arange(CW) // DH) * DH  # head base offsets
    perm = permC + perm1[np.arange(CW) % DH]

    scale = 1.0 / math.sqrt(DH)
    cos = np.concatenate([fc[:, :, 0].T, fc[:, :, 0].T])   # [128, S]
    sinp = np.concatenate([-fc[:, :, 1].T, fc[:, :, 1].T])  # [-sin; +sin]
    cqs = np.ascontiguousarray(cos * scale).astype(BF)
    sqs = np.ascontiguousarray(sinp * scale).astype(BF)
    cks = np.ascontiguousarray(cos).astype(BF)
    sks = np.ascontiguousarray(sinp).astype(BF)

    # mask M[k,q] = NEG where k > q; the kernel adds M via ss += lhsT.T @ I
    # with lhsT = M^T
    tri = np.where(np.arange(128)[:, None] > np.arange(128)[None, :],
                   np.float32(NEG), np.float32(0.0))
    triT = np.ascontiguousarray(tri.T).astype(BF)
    ident = np.eye(128, dtype=np.float32).astype(BF)

    hTb = [np.ascontiguousarray(hs[b].T).astype(BF) for b in range(B)]

    in_maps = []
    for c in range(NCORES):
        b, hg = divmod(c, GPC)
        sl = slice(CW * hg, CW * (hg + 1))
        wq_s = Wq[sl][perm]
        wk_s = Wk[sl][perm]
        m = {
            "hiddenT": hTb[b],
            "wqT": np.ascontiguousarray(wq_s.T).astype(BF),
            "wkT": np.ascontiguousarray(wk_s.T).astype(BF),
            "wvT": np.ascontiguousarray(Wv[sl].T).astype(BF),
            "wopT": np.ascontiguousarray(Wo[:, sl].T).astype(BF),
            "cqs": cqs, "sqs": sqs, "cks": cks, "sks": sks,
            "bqp": np.ascontiguousarray(
                bq[sl][perm].reshape(HPC, 128).T).astype(np.float32),
            "bkp": np.ascontiguousarray(
                bk[sl][perm].reshape(HPC, 128).T).astype(np.float32),
            "dmaskT": triT,
            "ident": ident,
        }
        in_maps.append(m)
    return in_maps


def _is_causal(mask):
    mask = np.asarray(mask, np.float32)
    if mask.shape != (1, 1, S, S):
        return False
    m = mask[0, 0]
    expect = np.triu(np.full((S, S), np.float32(NEG)), k=1)
    return bool(np.array_equal(m, expect))


def run_on_cores(inputs, trace=False):
    """Compile+run; returns BassKernelResults."""
    from concourse.bass_utils import run_bass_kernel_spmd
    assert _is_causal(inputs["mask"]), "kernel supports the causal mask only"
    nc = _get_built()
    in_maps = _prep_inputs(inputs)
    r = run_bass_kernel_spmd(nc, in_maps, list(range(NCORES)), trace=trace)
    return r


def assemble(results, inputs):
    """Sum per-core partial outputs and fold in the bv/bo biases."""
    Wo = np.asarray(inputs["Wo"], np.float32)
    bv = np.asarray(inputs["bv"], np.float32)
    bo = np.asarray(inputs["bo"], np.float32)
    out = np.empty((B, S, D), np.float32)
    for b in range(B):
        acc = results[GPC * b]["pout"].T.astype(np.float32)
        for hg in range(1, GPC):
            acc = acc + results[GPC * b + hg]["pout"].T.astype(np.float32)
        out[b] = acc
    out += (bv @ Wo.T + bo)[None, None, :]
    return out


def kernel(**inputs) -> np.ndarray:
    r = run_on_cores(inputs)
    return assemble(r.results, inputs)


# revision 27
# speedup vs baseline: 1.0027x; 1.0027x over previous
"""Self-contained Trainium2 Bass kernel: 16-head attention with RoPE (B=2, S=2048, D=2048).

Sharding: 8 cores = 2 (batch) x 4 (head groups of 4 heads / 512 cols).
No collectives: the output projection is row-split (each core multiplies its
own 4 heads' attention output X_g by Wo rows for those columns) and emits a
full-width PARTIAL output [D, S] bf16; the host sums the 4 partials per batch.

The kernel is a software pipeline over 4 q-chunks of 512. Emission interleaves
the previous chunk's output-projection groups into the current chunk's
projection phase and attention rounds so the PE never starves while the
exp/rowsum chain (ACT/DVE) runs.

Dataflow is fully "transposed" so no on-chip transposes are needed:
  hiddenT [d, s] (host-pretransposed, bf16), streamed per chunk
  QT/KT   [dh, s] per head  (projection emits head-dim-major directly)
  S^T     [k, q] scores, two heads packed in one [128, 2, 512] PSUM tile
  causal mask on diagonal tiles applied INSIDE the score accumulation group
          via an extra matmul: ss += triT.T @ I  (no vector op on that path)
  P^T     [k, q] = exp(S^T)            (one ACT exp per head-pair per k-tile)
  colsums accumulated on DVE (bf16 2x), partition-reduced by a ones-matmul
  O^T     [dh, q] = V^T @ P^T          (lhsT = V natural [s, dh])
  partial out^T [oc, q] = Wo[:, own].T @ O^T  (accumulated over own heads)
RoPE de-interleave is folded into a host-side row permutation of Wq/Wk;
RoPE itself is 3 bf16 2x-mode DVE ops using [cos;cos] / [-sin;sin] tables.
1/sqrt(DH) is folded into the Q rope tables. bq/bk applied via ACT bias
(per-partition); bv/bo folded into the output on the host.
"""

import math
from contextlib import ExitStack

import numpy as np
import ml_dtypes

B, S, D, H, DH = 2, 2048, 2048, 16, 128
NCORES = 8
GPC = 4            # cores per head-group dimension
HPC = H // GPC     # heads per core (4)
CW = HPC * DH      # 512 columns per core
NEG = -1e9
BF = ml_dtypes.bfloat16
QCH = 512          # q-chunk (moving free dim)
NQC = S // QCH     # 4
NDT = D // 128     # 16 d-tiles
NST = S // 128     # 16 s-tiles

_built = {}


def _build():
    import concourse.bass as bass
    import concourse.tile as tile
    from concourse import bacc, mybir

    f32, bf16 = mybir.dt.float32, mybir.dt.bfloat16
    EXP = mybir.ActivationFunctionType.Exp
    IDN = mybir.ActivationFunctionType.Identity
    CPY = mybir.ActivationFunctionType.Copy

    nc = bacc.Bacc("TRN2", target_bir_lowering=False, debug=False,
                   num_devices=NCORES)

    hT_d = nc.dram_tensor("hiddenT", [D, S], bf16, kind="ExternalInput")
    wq_d = nc.dram_tensor("wqT", [D, CW], bf16, kind="ExternalInput")
    wk_d = nc.dram_tensor("wkT", [D, CW], bf16, kind="ExternalInput")
    wv_d = nc.dram_tensor("wvT", [D, CW], bf16, kind="ExternalInput")
    wo_d = nc.dram_tensor("wopT", [CW, D], bf16, kind="ExternalInput")
    cq_d = nc.dram_tensor("cqs", [128, S], bf16, kind="ExternalInput")
    sq_d = nc.dram_tensor("sqs", [128, S], bf16, kind="ExternalInput")
    ck_d = nc.dram_tensor("cks", [128, S], bf16, kind="ExternalInput")
    sk_d = nc.dram_tensor("sks", [128, S], bf16, kind="ExternalInput")
    bq_d = nc.dram_tensor("bqp", [128, HPC], f32, kind="ExternalInput")
    bk_d = nc.dram_tensor("bkp", [128, HPC], f32, kind="ExternalInput")
    dm_d = nc.dram_tensor("dmaskT", [128, 128], bf16, kind="ExternalInput")
    id_d = nc.dram_tensor("ident", [128, 128], bf16, kind="ExternalInput")
    out_d = nc.dram_tensor("pout", [D, S], bf16, kind="ExternalOutput")

    with tile.TileContext(nc) as tc, ExitStack() as ctx:
        wp = ctx.enter_context(tc.tile_pool(name="wp", bufs=3 * NDT))
        wop = ctx.enter_context(tc.tile_pool(name="wop", bufs=HPC))
        hp = ctx.enter_context(tc.tile_pool(name="hp", bufs=2 * NDT))
        cst = ctx.enter_context(tc.tile_pool(name="cst", bufs=1))
        qkp = ctx.enter_context(tc.tile_pool(name="qkp", bufs=2 * HPC + 1))
        vp = ctx.enter_context(tc.tile_pool(name="vp", bufs=NST))
        rp = ctx.enter_context(tc.tile_pool(name="rp", bufs=4))
        ptp = ctx.enter_context(tc.tile_pool(name="ptp", bufs=8))
        rcp = ctx.enter_context(tc.tile_pool(name="rcp", bufs=2))
        otp = ctx.enter_context(tc.tile_pool(name="otp", bufs=10))
        ofp = ctx.enter_context(tc.tile_pool(name="ofp", bufs=4))
        ps_mm = ctx.enter_context(tc.tile_pool(name="ps_mm", bufs=2, space="PSUM"))
        ps_ss = ctx.enter_context(tc.tile_pool(name="ps_ss", bufs=2, space="PSUM"))
        ps_pv = ctx.enter_context(tc.tile_pool(name="ps_pv", bufs=2, space="PSUM"))

        # All bulk input loads go on the SP HWDGE queue: spreading them onto
        # the ACT queue delays the latency-critical projection drain copies
        # behind ~0.6us-per-trigger dispatch on the Scalar sequencer.
        def ldma(dst, src):
            nc.sync.dma_start(dst, src)

        # ---- first-needed data first: Wv + hT(chunk 0) interleaved ----
        wv_sb, wq_sb, wk_sb, wo_sb = [], [], [], []
        hTc0 = []
        # Startup dispatch is ~0.6us/trigger serial per queue, so split the
        # two initial streams across BOTH HWDGE queues (ACT is idle until the
        # first projection drain at ~20us).
        for dt in range(NDT):
            w = wp.tile([128, CW], bf16, tag="w", name=f"wv{dt}")
            eng = nc.sync if dt % 2 == 0 else nc.scalar
            eng.dma_start(w[:], wv_d[dt * 128:(dt + 1) * 128, :])
            wv_sb.append(w)
            t = hp.tile([128, QCH], bf16, tag="hT", name=f"hT0_{dt}")
            eng2 = nc.scalar if dt % 2 == 0 else nc.sync
            eng2.dma_start(t[:], hT_d[dt * 128:(dt + 1) * 128, 0:QCH])
            hTc0.append(t)
        # ---- constants / Q then K weights ----
        cq_sb = cst.tile([128, S], bf16, tag="cq", name="cq_sb")
        sq_sb = cst.tile([128, S], bf16, tag="sq", name="sq_sb")
        ck_sb = cst.tile([128, S], bf16, tag="ck", name="ck_sb")
        sk_sb = cst.tile([128, S], bf16, tag="sk", name="sk_sb")
        bq_sb = cst.tile([128, HPC], f32, tag="bq", name="bq_sb")
        bk_sb = cst.tile([128, HPC], f32, tag="bk", name="bk_sb")
        for dt in range(NDT):
            w = wp.tile([128, CW], bf16, tag="w", name=f"wq{dt}")
            ldma(w[:], wq_d[dt * 128:(dt + 1) * 128, :])
            wq_sb.append(w)
        ldma(cq_sb[:], cq_d[:])
        ldma(sq_sb[:], sq_d[:])
        ldma(bq_sb[:], bq_d[:])
        for dt in range(NDT):
            w = wp.tile([128, CW], bf16, tag="w", name=f"wk{dt}")
            ldma(w[:], wk_d[dt * 128:(dt + 1) * 128, :])
            wk_sb.append(w)
        ldma(ck_sb[:], ck_d[:])
        ldma(sk_sb[:], sk_d[:])
        ldma(bk_sb[:], bk_d[:])
        triT_sb = cst.tile([128, 128], bf16, tag="triT", name="triT_sb")
        ldma(triT_sb[:], dm_d[:])
        id_sb = cst.tile([128, 128], bf16, tag="ident", name="id_sb")
        ldma(id_sb[:], id_d[:])
        ones_sb = cst.tile([128, 128], bf16, tag="ones", name="ones_sb")
        nc.vector.memset(ones_sb[:], 1.0)
        # PE warm-up: the HAM clock gate starts at 1.2 GHz and needs ~3.4us
        # of sustained activity to unthrottle. The PE is idle waiting on DMA
        # at kernel start anyway, so burn that time warming it up on data
        # that needs no DMA (results are discarded).
        warm_ps = ps_mm.tile([128, 64], f32, tag="mm", name="warm_ps")
        for i in range(24):
            nc.tensor.matmul(warm_ps[:], ones_sb[:], ones_sb[:, 0:64],
                             start=True, stop=True)
        # hT(1) after the chunk-0 weights: needed from ~50us so proj(1) can
        # fill attention(0) gaps, but must not delay wq/wk
        hTc1 = []
        for dt in range(NDT):
            t = hp.tile([128, QCH], bf16, tag="hT", name=f"hT1_{dt}")
            ldma(t[:], hT_d[dt * 128:(dt + 1) * 128, QCH:2 * QCH])
            hTc1.append(t)
        # Wo streams in behind everything else (needed first at outproj(0))
        for h in range(HPC):
            t = wop.tile([128, D], bf16, tag="wo", name=f"wo{h}")
            ldma(t[:], wo_d[h * 128:(h + 1) * 128, :])
            wo_sb.append(t)

        # persistent KT (written chunk by chunk; all history needed) and V;
        # QT is per-chunk only
        ktr = [qkp.tile([128, S], bf16, tag="ktr", name=f"ktr{m}", bufs=HPC)
               for m in range(HPC)]
        v_sb = [None] * NST
        hTcs = {0: hTc0, 1: hTc1}
        drain_flip = [0]

        def drain(dst, src):
            # alternate PSUM->SBUF drains between DVE and ACT so neither
            # engine serializes the psum slot recycling
            drain_flip[0] ^= 1
            if drain_flip[0]:
                nc.vector.tensor_copy(dst, src)
            else:
                nc.scalar.activation(dst, src, CPY)

        def rope_head(w_sb, b_sb, c_sb, s_sb, dst, dsl, hTc, c, m, prefix):
            """Project head m of chunk c and write RoPE'd rows to dst[:, dsl]."""
            csl = slice(c * QCH, (c + 1) * QCH)
            ps = ps_mm.tile([128, QCH], f32, tag="mm", name=f"{prefix}ps{m}_{c}")
            for dt in range(NDT):
                nc.tensor.matmul(ps[:], w_sb[dt][:, m * 128:(m + 1) * 128],
                                 hTc[dt][:],
                                 start=(dt == 0), stop=(dt == NDT - 1))
            raw = rp.tile([128, QCH], bf16, tag="raw", name=f"{prefix}rw{m}_{c}")
            # alternate the raw copy between ACT and DVE (both apply the
            # per-partition bias) so neither sequencer saturates
            drain_flip[0] ^= 1
            if drain_flip[0]:
                nc.scalar.activation(raw[:], ps[:], IDN, bias=b_sb[:, m:m + 1])
            else:
                nc.vector.tensor_scalar_add(raw[:], ps[:], b_sb[:, m:m + 1])
            t1 = rp.tile([128, QCH], bf16, tag="t1", name=f"{prefix}t1{m}_{c}")
            # half-swap on the ACT HWDGE queue: tiny latency-critical copies
            # must not sit behind bulk weight/hT transfers on the SP queue
            nc.scalar.dma_start(t1[0:64, :], raw[64:128, :])
            nc.scalar.dma_start(t1[64:128, :], raw[0:64, :])
            # dst = raw*[cos;cos] + swap(raw)*[-sin;sin]; the sin-multiply
            # runs on the otherwise-idle GPSIMD to decongest DVE
            nc.vector.tensor_mul(dst[:, dsl], raw[:], c_sb[:, csl])
            nc.gpsimd.tensor_mul(t1[:], t1[:], s_sb[:, csl])
            nc.vector.tensor_add(dst[:, dsl], dst[:, dsl], t1[:])

        def proj_tasks(c):
            """12 emission tasks: V s-tiles, Q heads (+rope), K heads (+rope)."""
            hTc = hTcs[c]
            tasks = []

            def v_task(sti):
                def go():
                    st = 4 * c + sti
                    ps = ps_mm.tile([128, CW], f32, tag="mm", name=f"psv{st}")
                    for dt in range(NDT):
                        nc.tensor.matmul(ps[:],
                                         hTc[dt][:, sti * 128:(sti + 1) * 128],
                                         wv_sb[dt][:],
                                         start=(dt == 0), stop=(dt == NDT - 1))
                    vt = vp.tile([128, CW], bf16, tag="v", name=f"v{st}")
                    drain(vt[:], ps[:])
                    v_sb[st] = vt
                return go

            qtrc = [qkp.tile([128, QCH], bf16, tag="qtc", name=f"qtc{c}_{m}",
                             bufs=HPC + 4) for m in range(HPC)]

            def q_task(m):
                return lambda: rope_head(wq_sb, bq_sb, cq_sb, sq_sb, qtrc[m],
                                         slice(0, QCH), hTc, c, m, "q")

            def k_task(m):
                return lambda: rope_head(wk_sb, bk_sb, ck_sb, sk_sb, ktr[m],
                                         slice(c * QCH, (c + 1) * QCH),
                                         hTc, c, m, "k")

            # pre: everything attention pass A (heads 0,1) needs.
            # mid: heads 2,3's projections — they become pass A's fillers so
            # their rope chains overlap pass A instead of gating it.
            for sti in range(4):
                tasks.append(v_task(sti))
            tasks.append(q_task(0))
            tasks.append(q_task(1))
            tasks.append(k_task(0))
            tasks.append(k_task(1))
            mid = [q_task(2), q_task(3), k_task(2), k_task(3)]
            return qtrc, tasks, mid

        def attention_pass(c, qtrc, pair, fillers):
            """Heads (2*pair, 2*pair+1) of chunk c; returns their ot tiles.
            Pops one filler emission task per k-tile round (if any left)."""
            nk = 4 * c + 4
            heads = (2 * pair, 2 * pair + 1)
            pv = {}
            for h in heads:
                pv[h] = ps_pv.tile([128, QCH], f32, tag="pv",
                                   name=f"pv{c}_{h}")
            sacc = ptp.tile([128, 2, QCH], bf16, tag="sacc",
                            name=f"sacc{c}_{pair}", bufs=3)
            prev_pt, prev_ki = None, None
            for ki in range(nk):
                p = ki - 4 * c
                c0 = max(0, 128 * p)
                ss = ps_ss.tile([128, 2, QCH], f32, tag="ss",
                                name=f"ss{c}_{pair}_{ki}")
                for j, h in enumerate(heads):
                    nc.tensor.matmul(ss[:, j, c0:],
                                     ktr[h][:, ki * 128:(ki + 1) * 128],
                                     qtrc[h][:, c0:],
                                     start=True, stop=(p < 0))
                    if p >= 0:
                        # causal mask inside the accumulation group:
                        # ss_band += triT.T @ I  (keeps ACT chain PE-only)
                        nc.tensor.matmul(ss[:, j, c0:c0 + 128], triT_sb[:],
                                         id_sb[:], start=False, stop=True)
                pt = ptp.tile([128, 2, QCH], bf16, tag="pt",
                              name=f"pt{c}_{pair}_{ki}", bufs=3)
                if c0 > 0:
                    nc.gpsimd.memset(pt[:, :, 0:c0], 0.0)
                nc.scalar.activation(pt[:, :, c0:], ss[:, :, c0:], EXP)
                if ki == 0:
                    nc.vector.tensor_copy(sacc[:], pt[:])
                else:
                    nc.vector.tensor_add(sacc[:, :, c0:], sacc[:, :, c0:],
                                         pt[:, :, c0:])
                # pv for the previous k-tile (one behind, so PE never waits
                # on the exp chain)
                if prev_pt is not None:
                    pc0 = max(0, 128 * (prev_ki - 4 * c))
                    for j, h in enumerate(heads):
                        nc.tensor.matmul(pv[h][:, pc0:],
                                         v_sb[prev_ki][:, h * 128:(h + 1) * 128],
                                         prev_pt[:, j, pc0:],
                                         start=(prev_ki == 0), stop=False)
                prev_pt, prev_ki = pt, ki
                if fillers:
                    fillers.pop(0)()
            fc0 = max(0, 128 * (prev_ki - 4 * c)) if prev_ki != 0 else 0
            for j, h in enumerate(heads):
                nc.tensor.matmul(pv[h][:, fc0:],
                                 v_sb[prev_ki][:, h * 128:(h + 1) * 128],
                                 prev_pt[:, j, fc0:],
                                 start=(prev_ki == 0), stop=True)
            # partition-reduce+broadcast the colsums (two 512-wide matmuls)
            sm = ps_ss.tile([128, 2, QCH], f32, tag="ss", name=f"sm{c}_{pair}")
            for j in range(2):
                nc.tensor.matmul(sm[:, j, :], ones_sb[:], sacc[:, j, :],
                                 start=True, stop=True)
            recb = rcp.tile([128, 2, QCH], f32, tag="recb",
                            name=f"recb{c}_{pair}")
            nc.vector.reciprocal_approx_fast(out=recb[:], in_=sm[:])
            ots = []
            for j, h in enumerate(heads):
                ot = otp.tile([128, QCH], bf16, tag="ot", name=f"ot{c}_{h}")
                nc.vector.tensor_mul(ot[:], pv[h][:], recb[:, j, :])
                ots.append(ot)
            return ots

        def outproj_tasks(c, ots, tail=False):
            """16 emission tasks, one [128,512] output tile each."""
            csl = slice(c * QCH, (c + 1) * QCH)

            def task(t):
                def go():
                    pool = ps_ss if (tail and t % 2) else ps_mm
                    po = pool.tile([128, QCH], f32,
                                   tag="ss" if (tail and t % 2) else "mm",
                                   name=f"po{c}_{t}")
                    for h in range(HPC):
                        nc.tensor.matmul(po[:],
                                         wo_sb[h][:, t * 128:(t + 1) * 128],
                                         ots[h][:],
                                         start=(h == 0), stop=(h == HPC - 1))
                    of = ofp.tile([128, QCH], bf16, tag="of", name=f"of{c}_{t}")
                    drain(of[:], po[:])
                    nc.sync.dma_start(out_d[t * 128:(t + 1) * 128, csl], of[:])
                return go
            return [task(t) for t in range(NDT)]

        # ---- main pipeline ----
        po_backlog = []   # outproj tasks of the previous chunk
        for c in range(NQC):
            if 1 <= c and c + 1 < NQC:
                nsl = slice((c + 1) * QCH, (c + 2) * QCH)
                nxt = []
                for dt in range(NDT):
                    t = hp.tile([128, QCH], bf16, tag="hT",
                                name=f"hT{c + 1}_{dt}")
                    ldma(t[:], hT_d[dt * 128:(dt + 1) * 128, nsl])
                    nxt.append(t)
                hTcs[c + 1] = nxt
            qtrc, ptasks, mid = proj_tasks(c)
            # interleave proj(c) groups with half of outproj(c-1) groups
            first_po = po_backlog[:8]
            rest_po = po_backlog[8:]
            k = 0
            for i, t in enumerate(ptasks):
                t()
                if k < len(first_po):
                    first_po[k]()
                    k += 1
            for t in first_po[k:]:
                t()
            # heads 2,3's projections + remaining outproj(c-1) groups fill
            # the attention rounds (pass A always has >= 4 rounds, so `mid`
            # is fully emitted before pass B needs those heads); hold a few
            # outproj groups back so pass B doesn't run dry
            fill_a = mid + rest_po[:4]
            fill_b = rest_po[4:]
            ots = attention_pass(c, qtrc, 0, fill_a)
            fill_b = fill_a + fill_b
            ots = ots + attention_pass(c, qtrc, 1, fill_b)
            for t in fill_b:
                t()
            po_backlog = outproj_tasks(c, ots, tail=(c == NQC - 1))
        for t in po_backlog:
            t()

    nc.compile()
    return nc


def _get_built():
    if "k" not in _built:
        _built["k"] = _build()
    return _built["k"]


def _prep_inputs(inputs):
    hs = np.asarray(inputs["hidden_states"], np.float32)
    fc = np.asarray(inputs["freqs_cis"], np.float32)
    Wq = np.asarray(inputs["Wq"], np.float32)
    Wk = np.asarray(inputs["Wk"], np.float32)
    Wv = np.asarray(inputs["Wv"], np.float32)
    Wo = np.asarray(inputs["Wo"], np.float32)
    bq = np.asarray(inputs["bq"], np.float32)
    bk = np.asarray(inputs["bk"], np.float32)

    # de-interleave permutation per 128-row head block: [0,2,..,126, 1,3,..,127]
    perm1 = np.concatenate([np.arange(0, DH, 2), np.arange(1, DH, 2)])
    permC = (np.arange(CW) // DH) * DH  # head base offsets
    perm = permC + perm1[np.arange(CW) % DH]

    scale = 1.0 / math.sqrt(DH)
    cos = np.concatenate([fc[:, :, 0].T, fc[:, :, 0].T])   # [128, S]
    sinp = np.concatenate([-fc[:, :, 1].T, fc[:, :, 1].T])  # [-sin; +sin]
    cqs = np.ascontiguousarray(cos * scale).astype(BF)
    sqs = np.ascontiguousarray(sinp * scale).astype(BF)
    cks = np.ascontiguousarray(cos).astype(BF)
    sks = np.ascontiguousarray(sinp).astype(BF)

    # mask M[k,q] = NEG where k > q; the kernel adds M via ss += lhsT.T @ I
    # with lhsT = M^T
    tri = np.where(np.arange(128)[:, None] > np.arange(128)[None, :],
                   np.float32(NEG), np.float32(0.0))
    triT = np.ascontiguousarray(tri.T).astype(BF)
    ident = np.eye(128, dtype=np.float32).astype(BF)

    hTb = [np.ascontiguousarray(hs[b].T).astype(BF) for b in range(B)]

    in_maps = []
    for c in range(NCORES):
        b, hg = divmod(c, GPC)
        sl = slice(CW * hg, CW * (hg + 1))
        wq_s = Wq[sl][perm]
        wk_s = Wk[sl][perm]
        m = {
            "hiddenT": hTb[b],
            "wqT": np.ascontiguousarray(wq_s.T).astype(BF),
            "wkT": np.ascontiguousarray(wk_s.T).astype(BF),
            "wvT": np.ascontiguousarray(Wv[sl].T).astype(BF),
            "wopT": np.ascontiguousarray(Wo[:, sl].T).astype(BF),
            "cqs": cqs, "sqs": sqs, "cks": cks, "sks": sks,
            "bqp": np.ascontiguousarray(
                bq[sl][perm].reshape(HPC, 128).T).astype(np.float32),
            "bkp": np.ascontiguousarray(
                bk[sl][perm].reshape(HPC, 128).T).astype(np.float32),
            "dmaskT": triT,
            "ident": ident,
        }
        in_maps.append(m)
    return in_maps


def _is_causal(mask):
    mask = np.asarray(mask, np.float32)
    if mask.shape != (1, 1, S, S):
        return False
    m = mask[0, 0]
    expect = np.triu(np.full((S, S), np.float32(NEG)), k=1)
    return bool(np.array_equal(m, expect))


def run_on_cores(inputs, trace=False):
    """Compile+run; returns BassKernelResults."""
    from concourse.bass_utils import run_bass_kernel_spmd
    assert _is_causal(inputs["mask"]), "kernel supports the causal mask only"
    nc = _get_built()
    in_maps = _prep_inputs(inputs)
    r = run_bass_kernel_spmd(nc, in_maps, list(range(NCORES)), trace=trace)
    return r


def assemble(results, inputs):
    """Sum per-core partial outputs and fold in the bv/bo biases."""
    Wo = np.asarray(inputs["Wo"], np.float32)
    bv = np.asarray(inputs["bv"], np.float32)
    bo = np.asarray(inputs["bo"], np.float32)
    out = np.empty((B, S, D), np.float32)
    for b in range(B):
        acc = results[GPC * b]["pout"].T.astype(np.float32)
        for hg in range(1, GPC):
            acc = acc + results[GPC * b + hg]["pout"].T.astype(np.float32)
        out[b] = acc
    out += (bv @ Wo.T + bo)[None, None, :]
    return out


def kernel(**inputs) -> np.ndarray:
    r = run_on_cores(inputs)
    return assemble(r.results, inputs)


# revision 28
# speedup vs baseline: 1.1749x; 1.1718x over previous
"""Self-contained Trainium2 Bass kernel: 16-head attention with RoPE (B=2, S=2048, D=2048).

Sharding: 8 cores = 2 (batch) x 4 (head groups of 4 heads / 512 cols).
No collectives: the output projection is row-split (each core multiplies its
own 4 heads' attention output X_g by Wo rows for those columns) and emits a
full-width PARTIAL output [D, S] bf16; the host sums the 4 partials per batch.

The kernel is a software pipeline over 4 q-chunks of 512. Emission interleaves
the previous chunk's output-projection groups into the current chunk's
projection phase and attention rounds so the PE never starves while the
exp/rowsum chain (ACT/DVE) runs.

Dataflow is fully "transposed" so no on-chip transposes are needed:
  hiddenT [d, s] (host-pretransposed, bf16), streamed per chunk
  QT/KT   [dh, s] per head  (projection emits head-dim-major directly)
  S^T     [k, q] scores, two heads packed in one [128, 2, 512] PSUM tile
  causal mask on diagonal tiles applied INSIDE the score accumulation group
          via an extra matmul: ss += triT.T @ I  (no vector op on that path)
  P^T     [k, q] = exp(S^T)            (one ACT exp per head-pair per k-tile)
  colsums accumulated on DVE (bf16 2x), partition-reduced by a ones-matmul
  O^T     [dh, q] = V^T @ P^T          (lhsT = V natural [s, dh])
  partial out^T [oc, q] = Wo[:, own].T @ O^T  (accumulated over own heads)
RoPE de-interleave is folded into a host-side row permutation of Wq/Wk;
RoPE itself is 3 bf16 2x-mode DVE ops using [cos;cos] / [-sin;sin] tables.
1/sqrt(DH) is folded into the Q rope tables. bq/bk applied via ACT bias
(per-partition); bv/bo folded into the output on the host.
"""

import math
from contextlib import ExitStack

import numpy as np
import ml_dtypes

B, S, D, H, DH = 2, 2048, 2048, 16, 128
NCORES = 8
GPC = 4            # cores per head-group dimension
HPC = H // GPC     # heads per core (4)
CW = HPC * DH      # 512 columns per core
NEG = -1e9
BF = ml_dtypes.bfloat16
QCH = 512          # q-chunk (moving free dim)
NQC = S // QCH     # 4
NDT = D // 128     # 16 d-tiles
NST = S // 128     # 16 s-tiles

_built = {}


def _build():
    import concourse.bass as bass
    import concourse.tile as tile
    from concourse import bacc, mybir

    f32, bf16 = mybir.dt.float32, mybir.dt.bfloat16
    EXP = mybir.ActivationFunctionType.Exp
    IDN = mybir.ActivationFunctionType.Identity
    CPY = mybir.ActivationFunctionType.Copy

    nc = bacc.Bacc("TRN2", target_bir_lowering=False, debug=False,
                   num_devices=NCORES)

    hT_d = nc.dram_tensor("hiddenT", [D, S], bf16, kind="ExternalInput")
    wq_d = nc.dram_tensor("wqT", [D, CW], bf16, kind="ExternalInput")
    wk_d = nc.dram_tensor("wkT", [D, CW], bf16, kind="ExternalInput")
    wv_d = nc.dram_tensor("wvT", [D, CW], bf16, kind="ExternalInput")
    wo_d = nc.dram_tensor("wopT", [CW, D], bf16, kind="ExternalInput")
    cq_d = nc.dram_tensor("cqs", [128, S], bf16, kind="ExternalInput")
    sq_d = nc.dram_tensor("sqs", [128, S], bf16, kind="ExternalInput")
    ck_d = nc.dram_tensor("cks", [128, S], bf16, kind="ExternalInput")
    sk_d = nc.dram_tensor("sks", [128, S], bf16, kind="ExternalInput")
    bq_d = nc.dram_tensor("bqp", [128, HPC], f32, kind="ExternalInput")
    bk_d = nc.dram_tensor("bkp", [128, HPC], f32, kind="ExternalInput")
    dm_d = nc.dram_tensor("dmaskT", [128, 128], bf16, kind="ExternalInput")
    id_d = nc.dram_tensor("ident", [128, 128], bf16, kind="ExternalInput")
    out_d = nc.dram_tensor("pout", [D, S], bf16, kind="ExternalOutput")

    with tile.TileContext(nc) as tc, ExitStack() as ctx:
        wp = ctx.enter_context(tc.tile_pool(name="wp", bufs=3 * NDT))
        wop = ctx.enter_context(tc.tile_pool(name="wop", bufs=HPC))
        hp = ctx.enter_context(tc.tile_pool(name="hp", bufs=2 * NDT))
        cst = ctx.enter_context(tc.tile_pool(name="cst", bufs=1))
        qkp = ctx.enter_context(tc.tile_pool(name="qkp", bufs=2 * HPC + 1))
        vp = ctx.enter_context(tc.tile_pool(name="vp", bufs=NST))
        rp = ctx.enter_context(tc.tile_pool(name="rp", bufs=4))
        ptp = ctx.enter_context(tc.tile_pool(name="ptp", bufs=8))
        rcp = ctx.enter_context(tc.tile_pool(name="rcp", bufs=2))
        otp = ctx.enter_context(tc.tile_pool(name="otp", bufs=10))
        ofp = ctx.enter_context(tc.tile_pool(name="ofp", bufs=4))
        ps_mm = ctx.enter_context(tc.tile_pool(name="ps_mm", bufs=2, space="PSUM"))
        ps_ss = ctx.enter_context(tc.tile_pool(name="ps_ss", bufs=2, space="PSUM"))
        ps_pv = ctx.enter_context(tc.tile_pool(name="ps_pv", bufs=2, space="PSUM"))

        # All bulk input loads go on the SP HWDGE queue: spreading them onto
        # the ACT queue delays the latency-critical projection drain copies
        # behind ~0.6us-per-trigger dispatch on the Scalar sequencer.
        def ldma(dst, src):
            nc.sync.dma_start(dst, src)

        # ---- first-needed data first: Wv + hT(chunk 0) interleaved ----
        wv_sb, wq_sb, wk_sb, wo_sb = [], [], [], []
        hTc0 = []
        # Startup dispatch is ~0.6us/trigger serial per queue, so split the
        # two initial streams across BOTH HWDGE queues (ACT is idle until the
        # first projection drain at ~20us).
        for dt in range(NDT):
            w = wp.tile([128, CW], bf16, tag="w", name=f"wv{dt}")
            eng = nc.sync if dt % 2 == 0 else nc.scalar
            eng.dma_start(w[:], wv_d[dt * 128:(dt + 1) * 128, :])
            wv_sb.append(w)
            t = hp.tile([128, QCH], bf16, tag="hT", name=f"hT0_{dt}")
            eng2 = nc.scalar if dt % 2 == 0 else nc.sync
            eng2.dma_start(t[:], hT_d[dt * 128:(dt + 1) * 128, 0:QCH])
            hTc0.append(t)
        # ---- constants / Q then K weights ----
        cq_sb = cst.tile([128, S], bf16, tag="cq", name="cq_sb")
        sq_sb = cst.tile([128, S], bf16, tag="sq", name="sq_sb")
        ck_sb = cst.tile([128, S], bf16, tag="ck", name="ck_sb")
        sk_sb = cst.tile([128, S], bf16, tag="sk", name="sk_sb")
        bq_sb = cst.tile([128, HPC], f32, tag="bq", name="bq_sb")
        bk_sb = cst.tile([128, HPC], f32, tag="bk", name="bk_sb")
        for dt in range(NDT):
            w = wp.tile([128, CW], bf16, tag="w", name=f"wq{dt}")
            ldma(w[:], wq_d[dt * 128:(dt + 1) * 128, :])
            wq_sb.append(w)
        ldma(cq_sb[:], cq_d[:])
        ldma(sq_sb[:], sq_d[:])
        ldma(bq_sb[:], bq_d[:])
        for dt in range(NDT):
            w = wp.tile([128, CW], bf16, tag="w", name=f"wk{dt}")
            ldma(w[:], wk_d[dt * 128:(dt + 1) * 128, :])
            wk_sb.append(w)
        ldma(ck_sb[:], ck_d[:])
        ldma(sk_sb[:], sk_d[:])
        ldma(bk_sb[:], bk_d[:])
        triT_sb = cst.tile([128, 128], bf16, tag="triT", name="triT_sb")
        ldma(triT_sb[:], dm_d[:])
        id_sb = cst.tile([128, 128], bf16, tag="ident", name="id_sb")
        ldma(id_sb[:], id_d[:])
        ones_sb = cst.tile([128, 128], bf16, tag="ones", name="ones_sb")
        nc.vector.memset(ones_sb[:], 1.0)
        # PE warm-up: the HAM clock gate starts at 1.2 GHz and needs ~3.4us
        # of sustained activity to unthrottle. The PE is idle waiting on DMA
        # at kernel start anyway, so burn that time warming it up on data
        # that needs no DMA (results are discarded).
        warm_ps = ps_mm.tile([128, 64], f32, tag="mm", name="warm_ps")
        for i in range(24):
            nc.tensor.matmul(warm_ps[:], ones_sb[:], ones_sb[:, 0:64],
                             start=True, stop=True)
        # hT(1) after the chunk-0 weights: needed from ~50us so proj(1) can
        # fill attention(0) gaps, but must not delay wq/wk
        hTc1 = []
        for dt in range(NDT):
            t = hp.tile([128, QCH], bf16, tag="hT", name=f"hT1_{dt}")
            ldma(t[:], hT_d[dt * 128:(dt + 1) * 128, QCH:2 * QCH])
            hTc1.append(t)
        # Wo streams in behind everything else (needed first at outproj(0))
        for h in range(HPC):
            t = wop.tile([128, D], bf16, tag="wo", name=f"wo{h}")
            ldma(t[:], wo_d[h * 128:(h + 1) * 128, :])
            wo_sb.append(t)

        # persistent KT (written chunk by chunk; all history needed) and V;
        # QT is per-chunk only
        ktr = [qkp.tile([128, S], bf16, tag="ktr", name=f"ktr{m}", bufs=HPC)
               for m in range(HPC)]
        v_sb = [None] * NST
        hTcs = {0: hTc0, 1: hTc1}
        drain_flip = [0]

        def drain(dst, src):
            # alternate PSUM->SBUF drains between DVE and ACT so neither
            # engine serializes the psum slot recycling
            drain_flip[0] ^= 1
            if drain_flip[0]:
                nc.vector.tensor_copy(dst, src)
            else:
                nc.scalar.activation(dst, src, CPY)

        def rope_head(w_sb, b_sb, c_sb, s_sb, dst, dsl, hTc, c, m, prefix):
            """Project head m of chunk c and write RoPE'd rows to dst[:, dsl]."""
            csl = slice(c * QCH, (c + 1) * QCH)
            ps = ps_mm.tile([128, QCH], f32, tag="mm", name=f"{prefix}ps{m}_{c}")
            for dt in range(NDT):
                nc.tensor.matmul(ps[:], w_sb[dt][:, m * 128:(m + 1) * 128],
                                 hTc[dt][:],
                                 start=(dt == 0), stop=(dt == NDT - 1))
            raw = rp.tile([128, QCH], bf16, tag="raw", name=f"{prefix}rw{m}_{c}")
            # alternate the raw copy between ACT and DVE (both apply the
            # per-partition bias) so neither sequencer saturates
            drain_flip[0] ^= 1
            if drain_flip[0]:
                nc.scalar.activation(raw[:], ps[:], IDN, bias=b_sb[:, m:m + 1])
            else:
                nc.vector.tensor_scalar_add(raw[:], ps[:], b_sb[:, m:m + 1])
            t1 = rp.tile([128, QCH], bf16, tag="t1", name=f"{prefix}t1{m}_{c}")
            # half-swap on the ACT HWDGE queue: tiny latency-critical copies
            # must not sit behind bulk weight/hT transfers on the SP queue
            nc.scalar.dma_start(t1[0:64, :], raw[64:128, :])
            nc.scalar.dma_start(t1[64:128, :], raw[0:64, :])
            # dst = raw*[cos;cos] + swap(raw)*[-sin;sin]  (bf16 2x DVE ops;
            # GPSIMD was tried for the sin-multiply but is 3x slower per op,
            # which lengthens the rope chain that gates each attention pass)
            nc.vector.tensor_mul(dst[:, dsl], raw[:], c_sb[:, csl])
            nc.vector.tensor_mul(t1[:], t1[:], s_sb[:, csl])
            nc.vector.tensor_add(dst[:, dsl], dst[:, dsl], t1[:])

        def proj_tasks(c):
            """12 emission tasks: V s-tiles, Q heads (+rope), K heads (+rope)."""
            hTc = hTcs[c]
            tasks = []

            def v_task(sti):
                def go():
                    st = 4 * c + sti
                    ps = ps_mm.tile([128, CW], f32, tag="mm", name=f"psv{st}")
                    for dt in range(NDT):
                        nc.tensor.matmul(ps[:],
                                         hTc[dt][:, sti * 128:(sti + 1) * 128],
                                         wv_sb[dt][:],
                                         start=(dt == 0), stop=(dt == NDT - 1))
                    vt = vp.tile([128, CW], bf16, tag="v", name=f"v{st}")
                    drain(vt[:], ps[:])
                    v_sb[st] = vt
                return go

            qtrc = [qkp.tile([128, QCH], bf16, tag="qtc", name=f"qtc{c}_{m}",
                             bufs=HPC + 4) for m in range(HPC)]

            def q_task(m):
                return lambda: rope_head(wq_sb, bq_sb, cq_sb, sq_sb, qtrc[m],
                                         slice(0, QCH), hTc, c, m, "q")

            def k_task(m):
                return lambda: rope_head(wk_sb, bk_sb, ck_sb, sk_sb, ktr[m],
                                         slice(c * QCH, (c + 1) * QCH),
                                         hTc, c, m, "k")

            # pre: everything attention pass A (heads 0,1) needs.
            # mid: heads 2,3's projections — they become pass A's fillers so
            # their rope chains overlap pass A instead of gating it.
            for sti in range(4):
                tasks.append(v_task(sti))
            tasks.append(q_task(0))
            tasks.append(q_task(1))
            tasks.append(k_task(0))
            tasks.append(k_task(1))
            mid = [q_task(2), q_task(3), k_task(2), k_task(3)]
            return qtrc, tasks, mid

        def attention_pass(c, qtrc, pair, fillers):
            """Heads (2*pair, 2*pair+1) of chunk c; returns their ot tiles.
            Pops one filler emission task per k-tile round (if any left)."""
            nk = 4 * c + 4
            heads = (2 * pair, 2 * pair + 1)
            pv = {}
            for h in heads:
                pv[h] = ps_pv.tile([128, QCH], f32, tag="pv",
                                   name=f"pv{c}_{h}")
            sacc = ptp.tile([128, 2, QCH], bf16, tag="sacc",
                            name=f"sacc{c}_{pair}", bufs=3)
            prev_pt, prev_ki = None, None
            for ki in range(nk):
                p = ki - 4 * c
                c0 = max(0, 128 * p)
                ss = ps_ss.tile([128, 2, QCH], f32, tag="ss",
                                name=f"ss{c}_{pair}_{ki}")
                for j, h in enumerate(heads):
                    nc.tensor.matmul(ss[:, j, c0:],
                                     ktr[h][:, ki * 128:(ki + 1) * 128],
                                     qtrc[h][:, c0:],
                                     start=True, stop=(p < 0))
                    if p >= 0:
                        # causal mask inside the accumulation group:
                        # ss_band += triT.T @ I  (keeps ACT chain PE-only)
                        nc.tensor.matmul(ss[:, j, c0:c0 + 128], triT_sb[:],
                                         id_sb[:], start=False, stop=True)
                pt = ptp.tile([128, 2, QCH], bf16, tag="pt",
                              name=f"pt{c}_{pair}_{ki}", bufs=3)
                if c0 > 0:
                    nc.gpsimd.memset(pt[:, :, 0:c0], 0.0)
                nc.scalar.activation(pt[:, :, c0:], ss[:, :, c0:], EXP)
                if ki == 0:
                    nc.vector.tensor_copy(sacc[:], pt[:])
                else:
                    nc.vector.tensor_add(sacc[:, :, c0:], sacc[:, :, c0:],
                                         pt[:, :, c0:])
                # pv for the previous k-tile (one behind, so PE never waits
                # on the exp chain)
                if prev_pt is not None:
                    pc0 = max(0, 128 * (prev_ki - 4 * c))
                    for j, h in enumerate(heads):
                        nc.tensor.matmul(pv[h][:, pc0:],
                                         v_sb[prev_ki][:, h * 128:(h + 1) * 128],
                                         prev_pt[:, j, pc0:],
                                         start=(prev_ki == 0), stop=False)
                prev_pt, prev_ki = pt, ki
                if fillers:
                    fillers.pop(0)()
            fc0 = max(0, 128 * (prev_ki - 4 * c)) if prev_ki != 0 else 0
            for j, h in enumerate(heads):
                nc.tensor.matmul(pv[h][:, fc0:],
                                 v_sb[prev_ki][:, h * 128:(h + 1) * 128],
                                 prev_pt[:, j, fc0:],
                                 start=(prev_ki == 0), stop=True)
            # partition-reduce+broadcast the colsums (two 512-wide matmuls)
            sm = ps_ss.tile([128, 2, QCH], f32, tag="ss", name=f"sm{c}_{pair}")
            for j in range(2):
                nc.tensor.matmul(sm[:, j, :], ones_sb[:], sacc[:, j, :],
                                 start=True, stop=True)
            recb = rcp.tile([128, 2, QCH], f32, tag="recb",
                            name=f"recb{c}_{pair}")
            nc.vector.reciprocal_approx_fast(out=recb[:], in_=sm[:])
            ots = []
            for j, h in enumerate(heads):
                ot = otp.tile([128, QCH], bf16, tag="ot", name=f"ot{c}_{h}")
                nc.vector.tensor_mul(ot[:], pv[h][:], recb[:, j, :])
                ots.append(ot)
            return ots

        def outproj_tasks(c, ots, tail=False):
            """16 emission tasks, one [128,512] output tile each."""
            csl = slice(c * QCH, (c + 1) * QCH)

            def task(t):
                def go():
                    pool = ps_ss if (tail and t % 2) else ps_mm
                    po = pool.tile([128, QCH], f32,
                                   tag="ss" if (tail and t % 2) else "mm",
                                   name=f"po{c}_{t}")
                    for h in range(HPC):
                        nc.tensor.matmul(po[:],
                                         wo_sb[h][:, t * 128:(t + 1) * 128],
                                         ots[h][:],
                                         start=(h == 0), stop=(h == HPC - 1))
                    of = ofp.tile([128, QCH], bf16, tag="of", name=f"of{c}_{t}")
                    drain(of[:], po[:])
                    nc.sync.dma_start(out_d[t * 128:(t + 1) * 128, csl], of[:])
                return go
            return [task(t) for t in range(NDT)]

        # ---- main pipeline ----
        po_backlog = []   # outproj tasks of the previous chunk
        for c in range(NQC):
            if 1 <= c and c + 1 < NQC:
                nsl = slice((c + 1) * QCH, (c + 2) * QCH)
                nxt = []
                for dt in range(NDT):
                    t = hp.tile([128, QCH], bf16, tag="hT",
                                name=f"hT{c + 1}_{dt}")
                    ldma(t[:], hT_d[dt * 128:(dt + 1) * 128, nsl])
                    nxt.append(t)
                hTcs[c + 1] = nxt
            qtrc, ptasks, mid = proj_tasks(c)
            # interleave proj(c) groups with half of outproj(c-1) groups
            first_po = po_backlog[:8]
            rest_po = po_backlog[8:]
            k = 0
            for i, t in enumerate(ptasks):
                t()
                if k < len(first_po):
                    first_po[k]()
                    k += 1
            for t in first_po[k:]:
                t()
            # heads 2,3's projections + remaining outproj(c-1) groups fill
            # the attention rounds (pass A always has >= 4 rounds, so `mid`
            # is fully emitted before pass B needs those heads); hold a few
            # outproj groups back so pass B doesn't run dry
            fill_a = mid + rest_po[:4]
            fill_b = rest_po[4:]
            ots = attention_pass(c, qtrc, 0, fill_a)
            fill_b = fill_a + fill_b
            ots = ots + attention_pass(c, qtrc, 1, fill_b)
            for t in fill_b:
                t()
            po_backlog = outproj_tasks(c, ots, tail=(c == NQC - 1))
        for t in po_backlog:
            t()

    nc.compile()
    return nc


def _get_built():
    if "k" not in _built:
        _built["k"] = _build()
    return _built["k"]


def _prep_inputs(inputs):
    hs = np.asarray(inputs["hidden_states"], np.float32)
    fc = np.asarray(inputs["freqs_cis"], np.float32)
    Wq = np.asarray(inputs["Wq"], np.float32)
    Wk = np.asarray(inputs["Wk"], np.float32)
    Wv = np.asarray(inputs["Wv"], np.float32)
    Wo = np.asarray(inputs["Wo"], np.float32)
    bq = np.asarray(inputs["bq"], np.float32)
    bk = np.asarray(inputs["bk"], np.float32)

    # de-interleave permutation per 128-row head block: [0,2,..,126, 1,3,..,127]
    perm1 = np.concatenate([np.arange(0, DH, 2), np.arange(1, DH, 2)])
    permC = (np.arange(CW) // DH) * DH  # head base offsets
    perm = permC + perm1[np.arange(CW) % DH]

    scale = 1.0 / math.sqrt(DH)
    cos = np.concatenate([fc[:, :, 0].T, fc[:, :, 0].T])   # [128, S]
    sinp = np.concatenate([-fc[:, :, 1].T, fc[:, :, 1].T])  # [-sin; +sin]
    cqs = np.ascontiguousarray(cos * scale).astype(BF)
    sqs = np.ascontiguousarray(sinp * scale).astype(BF)
    cks = np.ascontiguousarray(cos).astype(BF)
    sks = np.ascontiguousarray(sinp).astype(BF)

    # mask M[k,q] = NEG where k > q; the kernel adds M via ss += lhsT.T @ I
    # with lhsT = M^T
    tri = np.where(np.arange(128)[:, None] > np.arange(128)[None, :],
                   np.float32(NEG), np.float32(0.0))
    triT = np.ascontiguousarray(tri.T).astype(BF)
    ident = np.eye(128, dtype=np.float32).astype(BF)

    hTb = [np.ascontiguousarray(hs[b].T).astype(BF) for b in range(B)]

    in_maps = []
    for c in range(NCORES):
        b, hg = divmod(c, GPC)
        sl = slice(CW * hg, CW * (hg + 1))
        wq_s = Wq[sl][perm]
        wk_s = Wk[sl][perm]
        m = {
            "hiddenT": hTb[b],
            "wqT": np.ascontiguousarray(wq_s.T).astype(BF),
            "wkT": np.ascontiguousarray(wk_s.T).astype(BF),
            "wvT": np.ascontiguousarray(Wv[sl].T).astype(BF),
            "wopT": np.ascontiguousarray(Wo[:, sl].T).astype(BF),
            "cqs": cqs, "sqs": sqs, "cks": cks, "sks": sks,
            "bqp": np.ascontiguousarray(
                bq[sl][perm].reshape(HPC, 128).T).astype(np.float32),
            "bkp": np.ascontiguousarray(
                bk[sl][perm].reshape(HPC, 128).T).astype(np.float32),
            "dmaskT": triT,
            "ident": ident,
        }
        in_maps.append(m)
    return in_maps


def _is_causal(mask):
    mask = np.asarray(mask, np.float32)
    if mask.shape != (1, 1, S, S):
        return False
    m = mask[0, 0]
    expect = np.triu(np.full((S, S), np.float32(NEG)), k=1)
    return bool(np.array_equal(m, expect))


def run_on_cores(inputs, trace=False):
    """Compile+run; returns BassKernelResults."""
    from concourse.bass_utils import run_bass_kernel_spmd
    assert _is_causal(inputs["mask"]), "kernel supports the causal mask only"
    nc = _get_built()
    in_maps = _prep_inputs(inputs)
    r = run_bass_kernel_spmd(nc, in_maps, list(range(NCORES)), trace=trace)
    return r


def assemble(results, inputs):
    """Sum per-core partial outputs and fold in the bv/bo biases."""
    Wo = np.asarray(inputs["Wo"], np.float32)
    bv = np.asarray(inputs["bv"], np.float32)
    bo = np.asarray(inputs["bo"], np.float32)
    out = np.empty((B, S, D), np.float32)
    for b in range(B):
        acc = results[GPC * b]["pout"].T.astype(np.float32)
        for hg in range(1, GPC):
            acc = acc + results[GPC * b + hg]["pout"].T.astype(np.float32)
        out[b] = acc
    out += (bv @ Wo.T + bo)[None, None, :]
    return out


def kernel(**inputs) -> np.ndarray:
    r = run_on_cores(inputs)
    return assemble(r.results, inputs)
